# revision 23
# baseline (speedup 1.0000x reference)
"""ConcatCritic pair-grid MLP on 8 TRN2 NeuronCores (Bass/Tile).

Reference computation (B=512, DX=DY=128, H=512):
    hx = x @ W1[:, :128].T            # [B, H]
    hy = y @ W1[:, 128:].T            # [B, H]
    h  = relu(hx[:,None,:] + hy[None,:,:] + b1)       # [B, B, H]
    z  = relu(h @ W2.T + b2)                          # [B, B, H]
    out[i, j] = z[i, j, :] @ W3[0] + b3               # [B, B]

Sharding: rows of x (the i axis) split across 8 cores, 64 rows each;
weights + y replicated. Each core computes its [64, 512] slab of the
pair grid; the host concatenates. No collectives needed.

Per-core dataflow (contraction dims live on SBUF partitions throughout):
  setup:  PE-transpose x, W1, W2 slices; mm1 -> hxbT[h, i] (+b1), hyT[h, j]
  per i:  ACT  A[hb]   = relu(hyT[hb] + hxbT[:, i])        (4x [128,512])
          PE   pz[kb] += W2T[hb,kb].T @ A[hb]              (16 matmuls)
          DVE  zr[kb]  = max(pz[kb] + b2[kb], 0)           (4x)
  per 4 rows (i0..i3):
          PE   prow[32m, :] += W3[kb].T @ zr[i_m][kb]      (16 M=1 matmuls,
               column-tiled to partition strips 0/32/64/96 so 4 stream
               concurrently through separate XBUSes)
          ACT  orow[32m, :] = prow[32m, :] + b3 ; one DMA -> out[i0:i0+4]
"""

import os

import numpy as np

import concourse.bass as bass
import concourse.bacc as bacc
import concourse.mybir as mybir
from concourse import tile
from concourse.masks import make_identity
from concourse.bass_utils import run_bass_kernel_spmd

B = 512
D = 128
H = 512
NCORES = 8
BI = B // NCORES  # 64 rows of x per core
HB = H // 128     # 4 h-blocks
KB = H // 128     # 4 k-blocks
G = 4             # rows per packed W3-reduce group
FP = mybir.dt.float32
Bb16 = mybir.dt.bfloat16
F16 = mybir.dt.float16
F32R = mybir.dt.float32r

Relu = mybir.ActivationFunctionType.Relu
Identity = mybir.ActivationFunctionType.Identity
Add = mybir.AluOpType.add
Max = mybir.AluOpType.max


def build(variant: str = "hybrid") -> bass.Bass:
    """variant — dtype of the pair-grid matmuls (mm2) / W3 reduce:
      'f32'    fp32 everywhere, serial reduce (slow; bit-exact baseline)
      'f32r'   f32r mm2 + f32r reduce (serial reduce — 4-byte ISA limit)
      'hybrid' f32r mm2 + fp16 reduce, 4-way packed at strips {0,32,64,96}
      'fp16'   fp16 everywhere, 4-way packed reduce
      'bf16'   bf16 everywhere, 4-way packed reduce
    Biases and PSUM stay fp32."""
    assert variant in ("f32", "f32r", "hybrid", "fp16", "bf16")
    # float32r data must be *written* by a compute op (engines round on
    # write) — walrus rejects plain-fp32-written data consumed by an FP32r
    # matmul. All mm operands below are produced by ACT/DVE/copy ops.
    mm_store = {"f32": FP, "f32r": F32R, "hybrid": F32R, "fp16": F16, "bf16": Bb16}[
        variant
    ]
    reduce_dt = {"f32": FP, "f32r": F32R, "hybrid": F16, "fp16": F16, "bf16": Bb16}[
        variant
    ]
    # Rows packed per W3-reduce psum bank. 4-byte (fp32/f32r) matmuls can
    # only write psum partition 0 (ISA 's3d3_mm_valid_dst_partition'), so
    # only 2-byte reduces can col-pack across partition strips.
    G = {"f32": 1, "f32r": 1, "hybrid": 4, "fp16": 4, "bf16": 4}[variant]
    stride = 128 // G
    # mm1 runs in f32r (4x cheaper than fp32, negligible error here)
    mm1_store = {"f32": FP, "f32r": F32R, "hybrid": F32R, "fp16": F16, "bf16": Bb16}[
        variant
    ]

    nc = bacc.Bacc(
        "TRN2",
        target_bir_lowering=False,
        debug=False,
        enable_asserts=False,
    )

    xs_d = nc.dram_tensor("xs", [BI, D], FP, kind="ExternalInput")
    y_d = nc.dram_tensor("y", [B, D], FP, kind="ExternalInput")
    W1_d = nc.dram_tensor("W1", [H, 2 * D], FP, kind="ExternalInput")
    b1_d = nc.dram_tensor("b1", [H], FP, kind="ExternalInput")
    W2_d = nc.dram_tensor("W2", [H, H], FP, kind="ExternalInput")
    b2_d = nc.dram_tensor("b2", [H], FP, kind="ExternalInput")
    W3_d = nc.dram_tensor("W3", [1, H], FP, kind="ExternalInput")
    b3_d = nc.dram_tensor("b3", [1], FP, kind="ExternalInput")
    out_d = nc.dram_tensor("out", [BI, B], FP, kind="ExternalOutput")

    with tile.TileContext(nc) as tc:
        with (
            tc.tile_pool(name="consts", bufs=1) as consts,
            tc.tile_pool(name="persist", bufs=1) as persist,
            tc.tile_pool(name="load", bufs=1) as load,
            tc.tile_pool(name="work", bufs=8) as work,
            tc.tile_pool(name="ps", bufs=6, space="PSUM") as ps,
        ):
            # ---------------- PE warm-up ----------------
            # The HAM clock gate needs ~3.4us of sustained PE activity to
            # lift the PE from 1.2 to 2.4 GHz. Burn dummy matmuls on scratch
            # data while the input DMAs are in flight so the real work runs
            # warm from the first transpose.
            warm_src = consts.tile([128, B], mm_store, name="warm_src")
            nc.vector.memset(warm_src, 0.0)
            warm_ps = ps.tile([128, B], FP, tag="bank", name="warm_ps")
            for _ in range(14):
                nc.tensor.matmul(
                    warm_ps, warm_src[:, :128], warm_src, start=True, stop=True
                )

            # identity before the gpsimd DMAs — every transpose needs it
            ident = consts.tile([128, 128], FP, name="ident")
            make_identity(nc, ident)

            # ---------------- bulk input DMAs (two queues) ----------------
            # xs (tiny) then W2 on sync; W2 gates the 16-transpose chain.
            xs_sb = load.tile([BI, D], FP, tag="xs_sb", name="xs_sb")
            nc.sync.dma_start(xs_sb, xs_d[:, :])
            w2_sb = load.tile([128, KB, H], FP, tag="w2_sb", name="w2_sb")
            nc.sync.dma_start(w2_sb, W2_d[:].rearrange("(kb p) h -> p kb h", p=128))
            w1_sb = load.tile([128, HB, 2 * D], FP, tag="w1_sb", name="w1_sb")
            nc.gpsimd.dma_start(
                w1_sb, W1_d[:].rearrange("(hb p) d -> p hb d", p=128)
            )
            y_sb = load.tile([128, B // 128, D], FP, tag="y_sb", name="y_sb")
            nc.gpsimd.dma_start(y_sb, y_d[:].rearrange("(jb p) d -> p jb d", p=128))

            b1c = consts.tile([128, HB], FP, name="b1c")
            nc.gpsimd.dma_start(b1c, b1_d[:].rearrange("(a p) -> p a", p=128))
            b2c = consts.tile([128, KB], FP, name="b2c")
            nc.gpsimd.dma_start(b2c, b2_d[:].rearrange("(a p) -> p a", p=128))
            w3c = consts.tile([128, KB], FP, name="w3c")
            nc.gpsimd.dma_start(w3c, W3_d[0].rearrange("(a p) -> p a", p=128))
            b3c = consts.tile([1, 1], FP, name="b3c")
            nc.gpsimd.dma_start(b3c, b3_d[None, :])
            ones1 = consts.tile([1, 128], FP, name="ones1")
            nc.vector.memset(ones1, 1.0)

            if variant == "f32":
                w3m = w3c
            else:
                w3m = consts.tile([128, KB], reduce_dt, name="w3m")
                nc.vector.tensor_copy(w3m, w3c)

            # b3 broadcast to all 128 partitions via a K=1 matmul
            b3_ps = ps.tile([128, 1], FP, tag="bank", name="b3_ps")
            nc.tensor.matmul(b3_ps, ones1, b3c, start=True, stop=True)
            b3bc = consts.tile([128, 1], FP, name="b3bc")
            nc.vector.tensor_copy(b3bc, b3_ps)

            # ---------------- transposes (PE) ----------------
            xsT = persist.tile([128, BI], mm1_store, name="xsT")
            t_ps = ps.tile([128, 128], FP, tag="bank", name="t_ps_x")
            nc.tensor.transpose(t_ps[:, :BI], xs_sb, ident[:BI, :BI])
            nc.vector.tensor_copy(xsT, t_ps[:, :BI])

            w1xT = []
            w1yT = []
            for hb in range(HB):
                tx = persist.tile([128, 128], mm1_store, name=f"w1xT{hb}")
                ty = persist.tile([128, 128], mm1_store, name=f"w1yT{hb}")
                px = ps.tile([128, 128], FP, tag="bank", name=f"t_ps_w1x{hb}")
                nc.tensor.transpose(px, w1_sb[:, hb, :D], ident)
                nc.vector.tensor_copy(tx, px)
                py = ps.tile([128, 128], FP, tag="bank", name=f"t_ps_w1y{hb}")
                nc.tensor.transpose(py, w1_sb[:, hb, D:], ident)
                nc.vector.tensor_copy(ty, py)
                w1xT.append(tx)
                w1yT.append(ty)

            yT = persist.tile([128, B], mm1_store, name="yT")
            for jb in range(B // 128):
                pj = ps.tile([128, 128], FP, tag="bank", name=f"t_ps_y{jb}")
                nc.tensor.transpose(pj, y_sb[:, jb, :], ident)
                nc.vector.tensor_copy(yT[:, jb * 128 : (jb + 1) * 128], pj)

            # ---------------- mm1: hxbT [h, i] (+b1), hyT [h, j] ----------------
            hxbT = persist.tile([128, HB * BI], FP, name="hxbT")
            hyT = [persist.tile([128, B], FP, name=f"hyT{hb}") for hb in range(HB)]
            for hb in range(HB):
                hx_ps = ps.tile([128, BI], FP, tag="bank", name=f"hx_ps{hb}")
                nc.tensor.matmul(hx_ps, w1xT[hb], xsT, start=True, stop=True)
                nc.scalar.activation(
                    hxbT[:, hb * BI : (hb + 1) * BI],
                    hx_ps,
                    Identity,
                    bias=b1c[:, hb : hb + 1],
                )
                hy_ps = ps.tile([128, B], FP, tag="bank", name=f"hy_ps{hb}")
                nc.tensor.matmul(hy_ps, w1yT[hb], yT, start=True, stop=True)
                nc.vector.tensor_copy(hyT[hb], hy_ps)

            # W2 transposes last on PE: mm1 (which gates A-gen) runs first,
            # and the first pair-grid matmul only needs the kb=0 slices.
            # W2T[hb] is [128 (h in block), 512 (k)]: W2T[hb][h, k] = W2[k, h]
            w2T = [
                persist.tile([128, H], mm_store, name=f"w2T{hb}") for hb in range(HB)
            ]
            for kb in range(KB):
                for hb in range(HB):
                    pw = ps.tile([128, 128], FP, tag="bank", name=f"t_ps_w2_{kb}_{hb}")
                    nc.tensor.transpose(
                        pw, w2_sb[:, kb, hb * 128 : (hb + 1) * 128], ident
                    )
                    nc.vector.tensor_copy(
                        w2T[hb][:, kb * 128 : (kb + 1) * 128], pw
                    )

            # ---------------- main loop over the 64 rows ----------------
            def emit_head(i):
                # A[hb] = relu(hyT[hb] + hx[i, :] + b1)   (ACT, per-part bias)
                A = [
                    work.tile([128, B], mm_store, tag="A", name=f"A{i}_{hb}")
                    for hb in range(HB)
                ]
                for hb in range(HB):
                    nc.scalar.activation(
                        A[hb],
                        hyT[hb],
                        Relu,
                        bias=hxbT[:, hb * BI + i : hb * BI + i + 1],
                    )
                # pz[kb] = sum_hb W2T[hb][:, kb].T @ A[hb]; zr = max(pz+b2, 0)
                zr = []
                for kb in range(KB):
                    pz = ps.tile([128, B], FP, tag="bank", name=f"pz{i}_{kb}")
                    for hb in range(HB):
                        nc.tensor.matmul(
                            pz,
                            w2T[hb][:, kb * 128 : (kb + 1) * 128],
                            A[hb],
                            start=(hb == 0),
                            stop=(hb == HB - 1),
                        )
                    z = work.tile(
                        [128, B],
                        reduce_dt,
                        tag="zr",
                        bufs=max(8, 8 * G),
                        name=f"zr{i}_{kb}",
                    )
                    nc.vector.tensor_scalar(
                        z, pz, b2c[:, kb : kb + 1], 0.0, Add, Max
                    )
                    zr.append(z)
                return zr

            def emit_tail(g, zrs):
                # out[4g+m, :] = sum_kb W3[kb].T @ zr[m][kb] + b3.
                # The 4 rows go to partitions 0/32/64/96 of ONE psum bank via
                # col-tiling, so the 4 M=1 matmuls of a kb round stream
                # concurrently. start/stop clear/mark has_written per written
                # element (not bank-wide), so each row chain carries its own.
                prow = ps.tile([128, B], FP, tag="prow", bufs=2, name=f"prow{g}")
                for kb in range(KB):
                    for m in range(G):
                        nc.tensor.matmul(
                            prow[stride * m : stride * m + 1, :],
                            w3m[:, kb : kb + 1],
                            zrs[m][kb],
                            start=(kb == 0),
                            stop=(kb == KB - 1),
                            tile_position=(0, stride * m),
                            skip_group_check=True,
                        )
                # One [128,512] ACT covers all G written rows (same cost as
                # [1,512] — lanes run in parallel); unwritten psum rows pass
                # through as junk and the DMA only picks the real rows.
                orow = work.tile([128, B], FP, tag="orow", bufs=2, name=f"orow{g}")
                nc.scalar.activation(orow, prow, Identity, bias=b3bc)
                nc.sync.dma_start(
                    out_d[g * G : (g + 1) * G, :],
                    orow.rearrange("(m r) j -> m r j", m=G)[:, 0, :],
                )

            # Software-pipeline: the packed reduce for group g is emitted
            # after the first row of group g+1, so the PE never waits on the
            # DVE relu of the rows it is reducing.
            pending = None
            cur = []
            for i in range(BI):
                cur.append(emit_head(i))
                if len(cur) == 1 and pending is not None:
                    emit_tail(i // G - 1, pending)
                if len(cur) == G:
                    pending, cur = cur, []
            emit_tail(BI // G - 1, pending)

    nc.compile()
    return nc


_BUILT: dict[str, bass.Bass] = {}


def _get_nc(variant: str) -> bass.Bass:
    if variant not in _BUILT:
        _BUILT[variant] = build(variant)
    return _BUILT[variant]


def run(inputs: dict, variant: str | None = None, trace: bool = False):
    """Returns (out [512, 512] float32, BassKernelResults)."""
    variant = variant or os.environ.get("CC_VARIANT", "fp16")
    nc = _get_nc(variant)
    x = np.ascontiguousarray(np.asarray(inputs["x"], dtype=np.float32))
    y = np.ascontiguousarray(np.asarray(inputs["y"], dtype=np.float32))
    W1 = np.ascontiguousarray(np.asarray(inputs["W1"], dtype=np.float32))
    b1 = np.ascontiguousarray(np.asarray(inputs["b1"], dtype=np.float32))
    W2 = np.ascontiguousarray(np.asarray(inputs["W2"], dtype=np.float32))
    b2 = np.ascontiguousarray(np.asarray(inputs["b2"], dtype=np.float32))
    W3 = np.ascontiguousarray(np.asarray(inputs["W3"], dtype=np.float32))
    b3 = np.ascontiguousarray(np.asarray(inputs["b3"], dtype=np.float32))
    in_maps = []
    for c in range(NCORES):
        in_maps.append(
            {
                "xs": np.ascontiguousarray(x[c * BI : (c + 1) * BI]),
                "y": y,
                "W1": W1,
                "b1": b1,
                "W2": W2,
                "b2": b2,
                "W3": W3,
                "b3": b3,
            }
        )
    res = run_bass_kernel_spmd(
        nc, in_maps, core_ids=list(range(NCORES)), trace=trace
    )
    out = np.concatenate([r["out"] for r in res.results], axis=0)
    return out, res


def kernel(**inputs) -> np.ndarray:
    out, _ = run(inputs)
    return out
